# revision 1
# baseline (speedup 1.0000x reference)
"""AsyncCKConv Trainium2 kernel — data-parallel over batch on 8 NeuronCores.

Reference computation (per batch b):
  feat/vals/times = x[...,0/1/2]
  tdn[t,n]   = (times[n] - pos[t]) / max(pos)
  h1[t,n,h]  = sin(om1*(W1f[feat[n],h] + tdn[t,n]*w1t[h] + b1[h]))
  h2[t,n,g]  = sin(om2*(h1 @ W2.T + b2))
  kern       = (h2 @ W3.T + b3) * keep[t,n],  keep = (times[n] <= pos[t])
  w_vals[n]  = vals[n] * cnt[n] / (C0 * S[n]),  S = sum_m same(n,m)*exp(-.5 sd^2)
  out[o,t]   = sum_n kern[t,n,o]*w_vals[n] + bias[o]
             = W3 @ s[:,t] + b3*c[t] + bias,  s[g,t] = sum_n wk*h2, c[t] = sum_n wk

Device layout: partition dim = (c,h) with c in 4 n-chunks of 64, h/g in 32.
Layer-1 arg is separable: arg1[(c,h),(t,nl)] = v[(c,h),t] + u[(c,h),nl].
The K=32 SIREN matmul runs full-width via blockdiag kron(I4, W2.T).
"""

import os
import sys

sys.path.insert(0, "/opt/trn_rl_repo")

import numpy as np

B, N, T, C, H, O = 32, 256, 128, 32, 32, 64
NCORES = 8
BPC = B // NCORES          # batches per core = 4
NCH = 4                    # n-chunks per batch (64 each)
NL = N // NCH              # 64
C0 = 0.3989422804014327
INV_C0 = 1.0 / C0

_CACHE: dict = {}


def _build_bass(reps: int = 1):
    import concourse.bass as bass
    import concourse.mybir as mybir
    from concourse import bacc, tile
    from concourse.alu_op_type import AluOpType as alu

    f32 = mybir.dt.float32
    bf16 = mybir.dt.bfloat16
    AFT = mybir.ActivationFunctionType
    AXX = mybir.AxisListType.X

    nc = bacc.Bacc(None, target_bir_lowering=False)

    # ---- DRAM parameters (per-core shard) ----
    times_e = nc.declare_dram_parameter("times", [BPC, N], f32, isOutput=False)
    vals_e = nc.declare_dram_parameter("vals", [BPC, N], f32, isOutput=False)
    feat_e = nc.declare_dram_parameter("feat", [BPC, N], f32, isOutput=False)
    ft4_e = nc.declare_dram_parameter("ft4", [BPC, 128, NL], f32, isOutput=False)
    pos_e = nc.declare_dram_parameter("positions", [1, T], f32, isOutput=False)
    w1t_e = nc.declare_dram_parameter("w1t", [128, 1], f32, isOutput=False)
    b1t_e = nc.declare_dram_parameter("b1t", [128, 1], f32, isOutput=False)
    b2t_e = nc.declare_dram_parameter("b2t", [128, 1], f32, isOutput=False)
    w2bd_e = nc.declare_dram_parameter("w2bd", [128, 128], f32, isOutput=False)
    w3t_e = nc.declare_dram_parameter("w3t", [H, O], f32, isOutput=False)
    b3r_e = nc.declare_dram_parameter("b3r", [1, O], f32, isOutput=False)
    biasc_e = nc.declare_dram_parameter("biasc", [O, 1], f32, isOutput=False)
    oms_e = nc.declare_dram_parameter("oms", [1, 2], f32, isOutput=False)
    dc4_e = nc.declare_dram_parameter("dc4", [128, 128], f32, isOutput=False)
    ti32_e = nc.declare_dram_parameter("ti32", [128, 128], f32, isOutput=False)
    out_e = nc.declare_dram_parameter("out", [BPC, O, T], f32, isOutput=True)

    with tile.TileContext(nc) as tc:
        with (
            tc.tile_pool(name="st", bufs=1) as st,
            tc.tile_pool(name="dens", bufs=4) as dens,
            tc.tile_pool(name="per_b", bufs=2) as per_b,
            tc.tile_pool(name="big", bufs=2) as big,
            tc.tile_pool(name="sl", bufs=3) as sl,
            tc.tile_pool(name="ps_bc", bufs=2, space="PSUM") as ps_sm,
            tc.tile_pool(name="ps_fin", bufs=1, space="PSUM") as ps_fin,
            tc.tile_pool(name="ps_mm", bufs=3, space="PSUM") as ps_mm,
            tc.tile_pool(name="dram", bufs=1, space="DRAM") as dram,
        ):
            # ---------- statics ----------
            pos_row = st.tile([1, T], f32)
            nc.sync.dma_start(pos_row[:], pos_e[:])
            w1t_t = st.tile([128, 1], f32)
            nc.sync.dma_start(w1t_t[:], w1t_e[:])
            b1_t = st.tile([128, 1], f32)
            nc.sync.dma_start(b1_t[:], b1t_e[:])
            b2_t = st.tile([128, 1], f32)
            nc.sync.dma_start(b2_t[:], b2t_e[:])
            w2bd_f = st.tile([128, 128], f32)
            nc.sync.dma_start(w2bd_f[:], w2bd_e[:])
            dc4_s = st.tile([128, 128], f32)
            nc.sync.dma_start(dc4_s[:], dc4_e[:])
            ti32_s = st.tile([128, 128], f32)
            nc.sync.dma_start(ti32_s[:], ti32_e[:])
            lhsT3 = st.tile([128, 128], f32)
            nc.vector.memset(lhsT3[:], 0.0)
            nc.sync.dma_start(lhsT3[0:H, 0:O], w3t_e[:])
            nc.sync.dma_start(lhsT3[H : H + 1, 0:O], b3r_e[:])
            bias_c = st.tile([O, 1], f32)
            nc.sync.dma_start(bias_c[:], biasc_e[:])

            ones128 = st.tile([128, 128], f32)
            nc.vector.memset(ones128[:], 0.0)
            nc.vector.memset(ones128[0:1, :], 1.0)
            zero_col = st.tile([128, 1], f32)
            nc.vector.memset(zero_col[:], 0.0)

            w2bd_b = st.tile([128, 128], bf16)
            nc.vector.tensor_copy(w2bd_b[:], w2bd_f[:])

            # scalars: [om1, om2, invP] -> broadcast to all partitions
            scal_rhs = st.tile([128, 3], f32)
            nc.vector.memset(scal_rhs[:], 0.0)
            nc.sync.dma_start(scal_rhs[0:1, 0:2], oms_e[:])
            pmax = st.tile([1, 1], f32)
            nc.vector.tensor_reduce(pmax[:], pos_row[:], AXX, alu.max)
            nc.vector.reciprocal(scal_rhs[0:1, 2:3], pmax[:])
            scal_ps = ps_sm.tile([128, 3], f32, tag="bc")
            nc.tensor.matmul(scal_ps[:], ones128[:], scal_rhs[:])
            scal_b = st.tile([128, 3], f32)
            nc.vector.tensor_copy(scal_b[:], scal_ps[:])
            om1_col = scal_b[:, 0:1]
            om2_col = scal_b[:, 1:2]
            invp_col = scal_b[:, 2:3]

            w1ts = st.tile([128, 1], f32)      # w1t * invP
            nc.vector.tensor_scalar(w1ts[:], w1t_t[:], invp_col, None, alu.mult)
            negw1ts = st.tile([128, 1], f32)
            nc.vector.tensor_scalar(negw1ts[:], w1ts[:], -1.0, None, alu.mult)
            b2om = st.tile([128, 1], f32)      # om2 * b2
            nc.vector.tensor_scalar(b2om[:], b2_t[:], om2_col, None, alu.mult)

            pos_col = st.tile([128, 1], f32)
            nc.sync.dma_start(pos_col[:], pos_e[0:1, :].rearrange("a (p q) -> (a p) q", q=1))
            # pos broadcast to all 128 partitions
            pos_rhs = st.tile([128, T], f32)
            nc.vector.memset(pos_rhs[:], 0.0)
            nc.vector.tensor_copy(pos_rhs[0:1, :], pos_row[:])
            posb_ps = ps_sm.tile([128, T], f32, tag="bc")
            nc.tensor.matmul(posb_ps[:], ones128[:], pos_rhs[:])
            pos_b = st.tile([128, T], f32)
            nc.vector.tensor_copy(pos_b[:], posb_ps[:])

            # v[(c,h), t] = -pos[t]*w1t[h]*invP
            vT4 = st.tile([128, T], f32)
            nc.vector.tensor_scalar(vT4[:], pos_b[:], negw1ts[:], None, alu.mult)

            wv_drams = [dram.tile([1, N], f32, name=f"wvd{i}") for i in range(BPC)]
            c_drams = [dram.tile([1, T], f32, name=f"cd{i}") for i in range(BPC)]

            for _rep in range(reps):
              def emit_density(b):
                  # ---------- density -> w_vals ----------
                  t_row = dens.tile([128, N], f32, tag="trow")
                  nc.vector.memset(t_row[:], 0.0)
                  nc.sync.dma_start(t_row[0:1, :], times_e[b : b + 1, :])
                  f_row = dens.tile([128, N], f32, tag="frow")
                  nc.vector.memset(f_row[:], 0.0)
                  nc.sync.dma_start(f_row[0:1, :], feat_e[b : b + 1, :])
                  tb_ps = ps_sm.tile([128, N], f32, tag="bc")
                  nc.tensor.matmul(tb_ps[:], ones128[:], t_row[:])
                  tb_full = dens.tile([128, N], f32, tag="tbf")
                  nc.vector.tensor_copy(tb_full[:], tb_ps[:])
                  fb_ps = ps_sm.tile([128, N], f32, tag="bc")
                  nc.tensor.matmul(fb_ps[:], ones128[:], f_row[:])
                  fb_full = dens.tile([128, N], f32, tag="fbf")
                  nc.vector.tensor_copy(fb_full[:], fb_ps[:])

                  for k in range(2):
                      nsl = slice(k * 128, k * 128 + 128)
                      t_col = dens.tile([128, 1], f32, tag="tcol")
                      nc.sync.dma_start(
                          t_col[:],
                          times_e[b : b + 1, nsl].rearrange("a (p q) -> (a p) q", q=1),
                      )
                      f_col = dens.tile([128, 1], f32, tag="fcol")
                      nc.sync.dma_start(
                          f_col[:],
                          feat_e[b : b + 1, nsl].rearrange("a (p q) -> (a p) q", q=1),
                      )
                      v_col = dens.tile([128, 1], f32, tag="vcol")
                      nc.sync.dma_start(
                          v_col[:],
                          vals_e[b : b + 1, nsl].rearrange("a (p q) -> (a p) q", q=1),
                      )
                      sd = dens.tile([128, N], f32, tag="sd")
                      nc.vector.tensor_scalar(sd[:], tb_full[:], t_col[:], None, alu.subtract)
                      sq = dens.tile([128, N], f32, tag="sq")
                      nc.vector.tensor_tensor(sq[:], sd[:], sd[:], alu.mult)
                      ek = dens.tile([128, N], f32, tag="ek")
                      nc.scalar.activation(ek[:], sq[:], AFT.Exp, bias=zero_col[:], scale=-0.5)
                      same = dens.tile([128, N], f32, tag="same")
                      nc.vector.tensor_scalar(same[:], fb_full[:], f_col[:], None, alu.is_equal)
                      masked = dens.tile([128, N], f32, tag="msk")
                      s_col = dens.tile([128, 1], f32, tag="scol")
                      nc.vector.scalar_tensor_tensor(
                          masked[:], ek[:], 1.0, same[:], alu.mult, alu.mult,
                          accum_out=s_col[:],
                      )
                      cnt = dens.tile([128, 1], f32, tag="cnt")
                      nc.vector.tensor_reduce(cnt[:], same[:], AXX, alu.add)
                      rec = dens.tile([128, 1], f32, tag="rec")
                      nc.vector.reciprocal(rec[:], s_col[:])
                      t1 = dens.tile([128, 1], f32, tag="t1")
                      nc.vector.tensor_tensor(t1[:], v_col[:], cnt[:], alu.mult)
                      wv_col = dens.tile([128, 1], f32, tag="wvc")
                      nc.vector.tensor_scalar(
                          wv_col[:], t1[:], rec[:], INV_C0, alu.mult, alu.mult
                      )
                      nc.sync.dma_start(
                          wv_drams[b][0:1, nsl].rearrange("a (p q) -> (a p) q", q=1),
                          wv_col[:],
                      )

                  # ---- c[t] = sum_m wv[m]*keep in [t, m] layout, fused ----
                  wvb_rhs = dens.tile([128, N], f32, tag="wvbr")
                  nc.vector.memset(wvb_rhs[:], 0.0)
                  nc.sync.dma_start(wvb_rhs[0:1, :], wv_drams[b][:])
                  wvb_ps = ps_sm.tile([128, N], f32, tag="bc")
                  nc.tensor.matmul(wvb_ps[:], ones128[:], wvb_rhs[:])
                  keep_t = dens.tile([128, N], f32, tag="keept")
                  nc.vector.tensor_scalar(keep_t[:], tb_full[:], pos_col[:], None, alu.is_le)
                  cjunk = dens.tile([128, N], f32, tag="cjunk")
                  c_col = dens.tile([128, 1], f32, tag="ccol")
                  nc.vector.tensor_tensor(cjunk[:], keep_t[:], wvb_ps[:], alu.mult)
                  nc.vector.tensor_reduce(c_col[:], cjunk[:], AXX, alu.add)
                  nc.sync.dma_start(
                      c_drams[b][0:1, :].rearrange("a (p q) -> (a p) q", q=1), c_col[:]
                  )

              def emit_main(b):
                  # w_vals broadcast into block layout: wv4[(c,j), nl] = wv[c*64+nl]
                  wv4_rhs = per_b.tile([128, NL], f32, tag="wv4r")
                  nc.vector.memset(wv4_rhs[:], 0.0)
                  nc.sync.dma_start(
                      wv4_rhs[0:NCH, :], wv_drams[b][:].rearrange("a (c n) -> (a c) n", n=NL)
                  )
                  wv4_ps = ps_sm.tile([128, NL], f32, tag="bc")
                  nc.tensor.matmul(wv4_ps[:], dc4_s[:], wv4_rhs[:])
                  wv4_b = per_b.tile([128, NL], bf16, tag="wv4")
                  nc.vector.tensor_copy(wv4_b[:], wv4_ps[:])

                  # times in block layout: tb128[(c,j), nl] = times[c*64+nl]
                  t4_rhs = per_b.tile([128, NL], f32, tag="t4r")
                  nc.vector.memset(t4_rhs[:], 0.0)
                  nc.sync.dma_start(
                      t4_rhs[0:NCH, :], times_e[b : b + 1, :].rearrange("a (c n) -> (a c) n", n=NL)
                  )
                  t4_ps = ps_sm.tile([128, NL], f32, tag="bc")
                  nc.tensor.matmul(t4_ps[:], dc4_s[:], t4_rhs[:])
                  tb128 = per_b.tile([128, NL], f32, tag="tb128")
                  nc.vector.tensor_copy(tb128[:], t4_ps[:])

                  # u[(c,h), nl] = ft4 + times*w1t*invP + b1
                  ft4_s = per_b.tile([128, NL], f32, tag="ft4")
                  nc.sync.dma_start(
                      ft4_s[:], ft4_e[b : b + 1].rearrange("a p n -> (a p) n")
                  )
                  uT4 = per_b.tile([128, NL], f32, tag="u")
                  nc.vector.tensor_scalar(uT4[:], tb128[:], w1ts[:], b1_t[:], alu.mult, alu.add)
                  nc.vector.tensor_tensor(uT4[:], uT4[:], ft4_s[:], alu.add)

                  s1 = per_b.tile([128, T], f32, tag="s1")

                  for th in range(2):
                      tsl = slice(th * 64, th * 64 + 64)
                      TF = 64 * NL  # 4096

                      arg1 = big.tile([128, TF], f32, tag="arg1", bufs=3)
                      nc.vector.tensor_tensor(
                          arg1[:].rearrange("p (t n) -> p t n", n=NL),
                          vT4[:, tsl].rearrange("p (t q) -> p t q", q=1).to_broadcast([128, 64, NL]),
                          uT4[:].rearrange("p (q n) -> p q n", q=1).to_broadcast([128, 64, NL]),
                          alu.add,
                      )
                      h1 = big.tile([128, TF], bf16, tag="h1", bufs=3)
                      nc.scalar.activation(h1[:], arg1[:], AFT.Sin, bias=zero_col[:], scale=om1_col)

                      keep = big.tile([128, TF], bf16, tag="keep")
                      nc.vector.tensor_tensor(
                          keep[:].rearrange("p (t n) -> p t n", n=NL),
                          tb128[:].rearrange("p (q n) -> p q n", q=1).to_broadcast([128, 64, NL]),
                          pos_b[:, tsl].rearrange("p (t q) -> p t q", q=1).to_broadcast([128, 64, NL]),
                          alu.is_le,
                      )
                      wk = big.tile([128, TF], bf16, tag="wk")
                      nc.vector.tensor_tensor(
                          wk[:].rearrange("p (t n) -> p t n", n=NL),
                          keep[:].rearrange("p (t n) -> p t n", n=NL),
                          wv4_b[:].rearrange("p (q n) -> p q n", q=1).to_broadcast([128, 64, NL]),
                          alu.mult,
                      )

                      h2f = big.tile([128, TF], bf16, tag="h2f")
                      for mm in range(8):
                          fsl = slice(mm * 512, mm * 512 + 512)
                          h2_ps = ps_mm.tile([128, 512], f32, tag="h2ps")
                          nc.tensor.matmul(h2_ps[:], w2bd_b[:], h1[:, fsl])
                          nc.scalar.activation(h2f[:, fsl], h2_ps[:], AFT.Sin, bias=b2om[:], scale=om2_col)
                      h2w = big.tile([128, TF], bf16, tag="h2w")
                      nc.vector.tensor_tensor(h2w[:], h2f[:], wk[:], alu.mult)
                      nc.vector.tensor_reduce(
                          s1[:, tsl],
                          h2w[:].rearrange("p (t n) -> p t n", n=NL),
                          AXX,
                          alu.add,
                      )

                  # ---------- final combine ----------
                  s_ps = ps_fin.tile([128, T], f32, tag="sps")
                  nc.tensor.matmul(s_ps[:], ti32_s[:], s1[:])
                  c_row = per_b.tile([1, T], f32, tag="crow")
                  nc.sync.dma_start(c_row[:], c_drams[b][:])
                  rhs3 = per_b.tile([128, T], f32, tag="rhs3")
                  nc.vector.memset(rhs3[:], 0.0)
                  nc.vector.tensor_copy(rhs3[0:H, :], s_ps[0:H, :])
                  nc.vector.tensor_copy(rhs3[H : H + 1, :], c_row[:])
                  out_ps = ps_fin.tile([128, T], f32, tag="ops")
                  nc.tensor.matmul(out_ps[:], lhsT3[:], rhs3[:])
                  out_s = per_b.tile([O, T], f32, tag="outs")
                  nc.vector.tensor_scalar(out_s[:], out_ps[0:O, :], bias_c[:], None, alu.add)
                  nc.sync.dma_start(out_e[b], out_s[:])

              for b in range(BPC):
                  emit_density(b)
              for b in range(BPC):
                  emit_main(b)

    nc.finalize()
    return nc


def _get_nc(reps: int = 1):
    key = ("nc", reps)
    if key not in _CACHE:
        _CACHE[key] = _build_bass(reps)
    return _CACHE[key]


def _prep_in_maps(x, positions, W1, b1, om1, W2, b2, om2, W3, b3, bias):
    x = np.asarray(x, np.float32)
    positions = np.asarray(positions, np.float32).reshape(1, T)
    W1 = np.asarray(W1, np.float32)
    b1 = np.asarray(b1, np.float32)
    W2 = np.asarray(W2, np.float32)
    b2 = np.asarray(b2, np.float32)
    W3 = np.asarray(W3, np.float32)
    b3 = np.asarray(b3, np.float32)
    bias = np.asarray(bias, np.float32).reshape(1, O)
    oms = np.array([[np.float32(om1), np.float32(om2)]], np.float32)

    feat_i = x[:, :, 0].astype(np.int32)
    vals = np.ascontiguousarray(x[:, :, 1])
    times = np.ascontiguousarray(x[:, :, 2])
    feat_f = np.ascontiguousarray(x[:, :, 0])

    # gather: ft4[b, c*32+h, nl] = W1[h, feat[b, c*64+nl]]
    w1f = W1[:, :C]                       # (H, C)
    ftg = w1f[:, feat_i]                  # (H, B, N)
    ftg = np.transpose(ftg, (1, 0, 2))    # (B, H, N)
    ft4 = np.empty((B, 128, NL), np.float32)
    for c in range(NCH):
        ft4[:, c * 32 : c * 32 + 32, :] = ftg[:, :, c * NL : (c + 1) * NL]

    w1t = np.tile(W1[:, C], NCH).reshape(128, 1)
    b1t = np.tile(b1, NCH).reshape(128, 1)
    b2t = np.tile(b2, NCH).reshape(128, 1)
    w2bd = np.kron(np.eye(NCH, dtype=np.float32), W2.T).astype(np.float32)
    w3t = np.ascontiguousarray(W3.T)      # (H, O)
    dc4 = np.zeros((128, 128), np.float32)
    dc4[0:NCH, :] = np.kron(np.eye(NCH, dtype=np.float32), np.ones((1, 32), np.float32))
    ti32 = np.zeros((128, 128), np.float32)
    ti32[:, 0:H] = np.tile(np.eye(H, dtype=np.float32), (NCH, 1))

    shared = dict(
        positions=positions, w1t=w1t, b1t=b1t, b2t=b2t, w2bd=w2bd, w3t=w3t,
        b3r=b3.reshape(1, O), biasc=bias.reshape(O, 1), oms=oms, dc4=dc4, ti32=ti32,
    )
    in_maps = []
    for i in range(NCORES):
        bs = slice(i * BPC, (i + 1) * BPC)
        m = dict(shared)
        m["times"] = np.ascontiguousarray(times[bs])
        m["vals"] = np.ascontiguousarray(vals[bs])
        m["feat"] = np.ascontiguousarray(feat_f[bs])
        m["ft4"] = np.ascontiguousarray(ft4[bs])
        in_maps.append(m)
    return in_maps


def run(inputs: dict, trace: bool = False):
    from concourse.bass_utils import run_bass_kernel_spmd

    nc = _get_nc()
    in_maps = _prep_in_maps(**inputs)
    res = run_bass_kernel_spmd(nc, in_maps, core_ids=list(range(NCORES)), trace=trace)
    out = np.concatenate([res.results[i]["out"] for i in range(NCORES)], axis=0)
    return out.astype(np.float32), res


def kernel(**inputs) -> np.ndarray:
    out, _ = run(inputs, trace=bool(int(os.environ.get("KERNEL_TRACE", "0"))))
    return out

